# revision 5
# baseline (speedup 1.0000x reference)
"""MultiHeadAttention Trainium2 kernel (8-core SPMD).

Problem: B=2, T=2048, C=1024, H=16 heads, D=64.
  out = softmax((q Wq^T + bq)(k Wk^T + bk)^T / sqrt(D)) (v Wv^T + bv) Wo^T + bo

Sharding: core c -> (batch b = c // 4, head-group g = c % 4).  Each core
computes 4 heads (a 256-wide slice of the projection space) of one batch
element, including its partial contribution to the row-sharded output
projection.  The host sums the 4 partial outputs per batch and adds bo.

v2 design vs v1:
  - Activations are pre-transposed AND pre-cast to bf16 on the HOST:
    xqt/xkt/xvt = x[b].T as [C, T] bf16.  No PE transposes, no PSUM->SBUF
    staging copies, and half the input DMA bytes.
  - Q/K projections: QT/KT[co, t] = W^T.T @ xT directly (bf16 matmul,
    fp32 psum), stored f32r with bias folded via DVE tensor_scalar_add.
  - V projected in NATURAL orientation (tokens on partitions):
    Vnat[t, ds] = xvT_tile.T @ Wv^T — feeds AV without any transpose.
    VN layout per key-tile: [4 heads x (64 V cols + 64 ones cols)]; the
    ones columns make each AV matmul also emit the softmax denominator.
  - S^T[k, q] per head pair: two row-packed matmuls (partitions 0:64 /
    64:128 -> concurrent PE row-groups) into one [128, 1024] psum pair;
    ONE exp activation covers both heads (bf16 out, scale=1/8 folded).
  - Output projection accumulates W_o slices over the 256 head dims,
    written bf16; bo is added on the host during the combine.
"""

import numpy as np

B, T, C, H, D = 2, 2048, 1024, 16, 64
NCORES = 8
GROUPS = 4              # head-groups == cores per batch element
HG = H // GROUPS        # heads per core
DS = HG * D             # per-core projection slice width (256)
TCH = 512               # token chunk (psum bank = 512 fp32)
NTCH = T // TCH         # 4
NCC = C // 128          # 8 contraction chunks
NKT = T // 128          # 16 key tiles
SCALE = float(D) ** -0.5

_NC_CACHE = None


def _emit(ctx, tc, io):
    from concourse import mybir

    nc = tc.nc
    f32 = mybir.dt.float32
    f32r = mybir.dt.float32r
    bf16 = mybir.dt.bfloat16
    EXP = mybir.ActivationFunctionType.Exp

    persist = ctx.enter_context(tc.tile_pool(name="persist", bufs=1))

    def ptile(tag, shape, dt=f32):
        return persist.tile(shape, dt, tag=tag, name=tag)

    # --- persistent SBUF tensors ---------------------------------------
    wsb = {}
    for name in ("wq", "wk", "wv"):
        tiles = []
        for c in range(NCC):
            t_ = ptile(f"{name}{c}", [128, DS], bf16)
            nc.scalar.dma_start(t_[:], io[name][c * 128:(c + 1) * 128, :])
            tiles.append(t_)
        wsb[name] = tiles
    wot = []
    for dc in range(2):
        t_ = ptile(f"wot{dc}", [128, C], bf16)
        nc.scalar.dma_start(t_[:], io["wot"][dc * 128:(dc + 1) * 128, :])
        wot.append(t_)

    bias = {}
    for name in ("bqs", "bks"):
        t_ = ptile(name, [128, 2])
        nc.scalar.dma_start(
            t_[:], io[name].rearrange("(a p) o -> p (a o)", p=128))
        bias[name] = t_
    bvb = ptile("bvb", [128, DS])
    nc.scalar.dma_start(bvb[:], io["bvb"][:, :])

    QT = [ptile(f"qt{i}", [128, T], f32r) for i in range(2)]
    KT = [ptile(f"kt{i}", [128, T], f32r) for i in range(2)]
    # V natural, all 16 key tiles in one buffer; per key tile the layout is
    # [4 heads x (64 V cols + 64 ones cols)].  Ones prefilled via memset;
    # V columns overwritten when the V projection lands.
    VNB = ptile("vnb", [128, NKT * 512], bf16)
    nc.gpsimd.memset(VNB[:], 1.0)

    # --- stage A: load + project q, k, v -------------------------------
    with tc.tile_pool(name="xin", bufs=1) as xinp, \
         tc.tile_pool(name="projps", bufs=4, space="PSUM") as projps:

        xt = {}
        for i, name in enumerate(("xkt", "xvt", "xqt")):
            t_ = xinp.tile([128, NCC * T], bf16, tag=name, name=name)
            eng = (nc.sync, nc.scalar, nc.sync)[i]
            eng.dma_start(
                t_[:].rearrange("p (a t) -> p a t", a=NCC),
                io[name].rearrange("(a p) t -> p a t", p=128))
            xt[name] = t_

        def qk_proj(xname, wname, bname, XT, tci):
            xsb = xt[xname]
            for co in range(2):
                pj = projps.tile([128, TCH], f32, tag="proj", name="proj")
                for c in range(NCC):
                    nc.tensor.matmul(
                        pj[:],
                        lhsT=wsb[wname][c][:, co * 128:(co + 1) * 128],
                        rhs=xsb[:, c * T + tci * TCH:c * T + (tci + 1) * TCH],
                        start=(c == 0), stop=(c == NCC - 1))
                nc.vector.tensor_scalar_add(
                    XT[co][:, tci * TCH:(tci + 1) * TCH],
                    pj[:], bias[bname][:, co:co + 1])

        for tci in range(NTCH):
            qk_proj("xkt", "wk", "bks", KT, tci)

        # V natural: Vnat[t, ds] = sum_c xvT[c, t].T @ wv[c, ds]
        xv = xt["xvt"]
        bvb3 = bvb[:].rearrange("p (h d) -> p h d", h=HG)
        for tt in range(NKT):
            pv = projps.tile([128, DS], f32, tag="proj", name="vnat")
            for c in range(NCC):
                nc.tensor.matmul(
                    pv[:],
                    lhsT=xv[:, c * T + tt * 128:c * T + (tt + 1) * 128],
                    rhs=wsb["wv"][c][:],
                    start=(c == 0), stop=(c == NCC - 1))
            dst3 = VNB[:, tt * 512:(tt + 1) * 512].rearrange(
                "p (h c) -> p h c", h=HG)[:, :, 0:64]
            src3 = pv[:].rearrange("p (h d) -> p h d", h=HG)
            nc.vector.tensor_add(dst3, src3, bvb3)

        for tci in range(NTCH):
            qk_proj("xqt", "wq", "bqs", QT, tci)

    # --- stage B: attention + output projection ------------------------
    # S pool: 3 x [128, 1024] (6 banks) pipelines scores->exp->AV;
    # otps: 2 accumulator banks for the current head pair.
    with tc.tile_pool(name="sps", bufs=3, space="PSUM") as sps, \
         tc.tile_pool(name="otps", bufs=2, space="PSUM") as otps, \
         tc.tile_pool(name="expsb", bufs=4) as expsb, \
         tc.tile_pool(name="otsb", bufs=4) as otsbp, \
         tc.tile_pool(name="recsb", bufs=4) as recp, \
         tc.tile_pool(name="outsb", bufs=2) as outsbp:

        for qc in range(NTCH):
            qcols = slice(qc * TCH, (qc + 1) * TCH)
            ot_sb = []
            for pr in range(2):
                otp = [otps.tile([128, TCH], f32, tag="ot", name="ot")
                       for _ in range(2)]
                for kt in range(NKT):
                    first, last = kt == 0, kt == NKT - 1
                    S = sps.tile([128, 2 * TCH], f32, tag="s", name="s")
                    for hh in range(2):
                        rows = slice(hh * 64, (hh + 1) * 64)
                        nc.tensor.matmul(
                            S[:, hh * TCH:(hh + 1) * TCH],
                            lhsT=KT[pr][rows, kt * 128:(kt + 1) * 128],
                            rhs=QT[pr][rows, qcols],
                            start=True, stop=True)
                    es = expsb.tile([128, 2 * TCH], bf16, tag="es",
                                    name="es")
                    nc.scalar.activation(es[:], S[:], EXP, scale=SCALE)
                    for hh in range(2):
                        h = pr * 2 + hh
                        nc.tensor.matmul(
                            otp[hh][:, :],
                            lhsT=VNB[:, kt * 512 + h * 128:
                                     kt * 512 + (h + 1) * 128],
                            rhs=es[:, hh * TCH:(hh + 1) * TCH],
                            start=first, stop=last)
                # normalize: psum rows 64-127 hold the denominator
                osb = otsbp.tile([128, TCH], bf16, tag="otsb",
                                 name="otsb")
                for hh in range(2):
                    rec = recp.tile([64, TCH], f32, tag="rec", name="rec")
                    nc.vector.reciprocal(rec[:], otp[hh][64:128, :])
                    nc.vector.tensor_mul(
                        osb[hh * 64:(hh + 1) * 64, :],
                        otp[hh][0:64, :], rec[:])
                ot_sb.append(osb)
            ob = outsbp.tile([128, NCC * TCH], bf16, tag="ob", name="ob")
            for ct in range(NCC):
                pp = sps.tile([128, TCH], f32, tag="s", name="prj")
                for dc in range(2):
                    nc.tensor.matmul(
                        pp[:],
                        lhsT=wot[dc][:, ct * 128:(ct + 1) * 128],
                        rhs=ot_sb[dc][:],
                        start=(dc == 0), stop=(dc == 1))
                nc.vector.tensor_copy(
                    ob[:, ct * TCH:(ct + 1) * TCH], pp[:])
            nc.sync.dma_start(
                io["out_t"][:, qcols].rearrange("(a p) t -> p a t", p=128),
                ob[:].rearrange("p (a t) -> p a t", a=NCC))


def build_nc(reps=1):
    from contextlib import ExitStack

    import concourse.tile as tile
    from concourse import bacc, mybir

    f32 = mybir.dt.float32
    bf16 = mybir.dt.bfloat16
    nc = bacc.Bacc("TRN2", target_bir_lowering=False, debug=False,
                   num_devices=NCORES)
    io = {}
    for name in ("xqt", "xkt", "xvt"):
        io[name] = nc.dram_tensor(name, [C, T], bf16,
                                  kind="ExternalInput").ap()
    for name in ("wq", "wk", "wv"):
        io[name] = nc.dram_tensor(name, [C, DS], bf16,
                                  kind="ExternalInput").ap()
    io["wot"] = nc.dram_tensor("wot", [DS, C], bf16, kind="ExternalInput").ap()
    for name in ("bqs", "bks"):
        io[name] = nc.dram_tensor(name, [DS, 1], f32, kind="ExternalInput").ap()
    io["bvb"] = nc.dram_tensor("bvb", [128, DS], f32, kind="ExternalInput").ap()
    io["out_t"] = nc.dram_tensor("out_t", [C, T], bf16,
                                 kind="ExternalOutput").ap()

    with tile.TileContext(nc) as tc:
        if reps == 1:
            with ExitStack() as ctx:
                _emit(ctx, tc, io)
        else:
            with tc.For_i(0, reps, 1):
                with ExitStack() as ctx:
                    _emit(ctx, tc, io)
    nc.compile()
    return nc


def get_nc():
    global _NC_CACHE
    if _NC_CACHE is None:
        _NC_CACHE = build_nc()
    return _NC_CACHE


def make_in_maps(q, k, v, Wq, bq, Wk, bk, Wv, bv, Wo, bo):
    import ml_dtypes

    bfdt = ml_dtypes.bfloat16
    q, k, v = (np.asarray(x, np.float32) for x in (q, k, v))
    Wq, Wk, Wv, Wo = (np.asarray(x, np.float32) for x in (Wq, Wk, Wv, Wo))
    bq, bk, bv, bo = (np.asarray(x, np.float32) for x in (bq, bk, bv, bo))
    # shared per-batch transposed activations (shared across 4 cores)
    xqt = [np.ascontiguousarray(q[b].T).astype(bfdt) for b in range(B)]
    xkt = [np.ascontiguousarray(k[b].T).astype(bfdt) for b in range(B)]
    xvt = [np.ascontiguousarray(v[b].T).astype(bfdt) for b in range(B)]
    in_maps = []
    for core in range(NCORES):
        b, g = divmod(core, GROUPS)
        sl = slice(g * DS, (g + 1) * DS)
        in_maps.append({
            "xqt": xqt[b],
            "xkt": xkt[b],
            "xvt": xvt[b],
            "wq": np.ascontiguousarray(Wq[sl, :].T).astype(bfdt),
            "wk": np.ascontiguousarray(Wk[sl, :].T).astype(bfdt),
            "wv": np.ascontiguousarray(Wv[sl, :].T).astype(bfdt),
            "wot": np.ascontiguousarray(Wo[:, sl].T).astype(bfdt),
            "bqs": np.ascontiguousarray(bq[sl].reshape(DS, 1)),
            "bks": np.ascontiguousarray(bk[sl].reshape(DS, 1)),
            "bvb": np.ascontiguousarray(
                np.broadcast_to(bv[sl], (128, DS))).astype(np.float32),
        })
    return in_maps


def combine(results, bo):
    out = np.zeros((B, T, C), np.float32)
    for core in range(NCORES):
        b, _ = divmod(core, GROUPS)
        out[b] += results[core]["out_t"].T.astype(np.float32)
    out += np.asarray(bo, np.float32)
    return out


def kernel(q, k, v, Wq, bq, Wk, bk, Wv, bv, Wo, bo):
    from concourse.bass_utils import run_bass_kernel_spmd

    nc = get_nc()
    in_maps = make_in_maps(q, k, v, Wq, bq, Wk, bk, Wv, bv, Wo, bo)
    res = run_bass_kernel_spmd(nc, in_maps, core_ids=list(range(NCORES)))
    return combine(res.results, bo)


# revision 13
# speedup vs baseline: 1.5401x; 1.5401x over previous
"""MultiHeadAttention Trainium2 kernel (8-core SPMD).

Problem: B=2, T=2048, C=1024, H=16 heads, D=64.
  out = softmax((q Wq^T + bq)(k Wk^T + bk)^T / sqrt(D)) (v Wv^T + bv) Wo^T + bo

Sharding: core c -> (batch b = c // 4, head-group g = c % 4).  Each core
computes 4 heads (a 256-wide slice of the projection space) of one batch
element, including its partial contribution to the row-sharded output
projection.  The host sums the 4 partial outputs per batch and adds bo.

v2 design vs v1:
  - Activations are pre-transposed AND pre-cast to bf16 on the HOST:
    xqt/xkt/xvt = x[b].T as [C, T] bf16.  No PE transposes, no PSUM->SBUF
    staging copies, and half the input DMA bytes.
  - Q/K projections: QT/KT[co, t] = W^T.T @ xT directly (bf16 matmul,
    fp32 psum), stored f32r with bias folded via DVE tensor_scalar_add.
  - V projected in NATURAL orientation (tokens on partitions):
    Vnat[t, ds] = xvT_tile.T @ Wv^T — feeds AV without any transpose.
    VN layout per key-tile: [4 heads x (64 V cols + 64 ones cols)]; the
    ones columns make each AV matmul also emit the softmax denominator.
  - S^T[k, q] per head pair: two row-packed matmuls (partitions 0:64 /
    64:128 -> concurrent PE row-groups) into one [128, 1024] psum pair;
    ONE exp activation covers both heads (bf16 out, scale=1/8 folded).
  - Output projection accumulates W_o slices over the 256 head dims,
    written bf16; bo is added on the host during the combine.
"""

import numpy as np

B, T, C, H, D = 2, 2048, 1024, 16, 64
NCORES = 8
GROUPS = 4              # head-groups == cores per batch element
HG = H // GROUPS        # heads per core
DS = HG * D             # per-core projection slice width (256)
TCH = 512               # token chunk (psum bank = 512 fp32)
NTCH = T // TCH         # 4
NCC = C // 128          # 8 contraction chunks
NKT = T // 128          # 16 key tiles
SCALE = float(D) ** -0.5

_NC_CACHE = None


def _emit(ctx, tc, io):
    from concourse import mybir

    nc = tc.nc
    f32 = mybir.dt.float32
    f32r = mybir.dt.float32r
    bf16 = mybir.dt.bfloat16
    EXP = mybir.ActivationFunctionType.Exp

    persist = ctx.enter_context(tc.tile_pool(name="persist", bufs=1))

    def ptile(tag, shape, dt=f32):
        return persist.tile(shape, dt, tag=tag, name=tag)

    # --- persistent SBUF tensors ---------------------------------------
    wsb = {}
    for name in ("wq", "wk", "wv"):
        tiles = []
        for c in range(NCC):
            t_ = ptile(f"{name}{c}", [128, DS], bf16)
            nc.scalar.dma_start(t_[:], io[name][c * 128:(c + 1) * 128, :])
            tiles.append(t_)
        wsb[name] = tiles
    wot = []
    for dc in range(2):
        t_ = ptile(f"wot{dc}", [128, C], bf16)
        nc.scalar.dma_start(t_[:], io["wot"][dc * 128:(dc + 1) * 128, :])
        wot.append(t_)

    bias = {}
    for name in ("bqs", "bks"):
        t_ = ptile(name, [128, 2])
        nc.scalar.dma_start(
            t_[:], io[name].rearrange("(a p) o -> p (a o)", p=128))
        bias[name] = t_
    bvb = ptile("bvb", [128, DS])
    nc.scalar.dma_start(bvb[:], io["bvb"][:, :])

    QT = [ptile(f"qt{i}", [128, T], f32r) for i in range(2)]
    KT = [ptile(f"kt{i}", [128, T], f32r) for i in range(2)]
    # V natural, all 16 key tiles in one buffer; per key tile the layout is
    # [4 heads x (64 V cols + 64 ones cols)].  Ones prefilled via memset;
    # V columns overwritten when the V projection lands.
    VNB = ptile("vnb", [128, NKT * 512], bf16)
    nc.gpsimd.memset(VNB[:], 1.0)

    # --- stage A + B, software-pipelined -------------------------------
    # DMA order: weights (scalar), xkt (sync), xqt (scalar), xvt (sync):
    # scores for q-chunk 0 only need K fully projected + Q chunk 0, so the
    # exp stream starts as soon as K + Q0 land; V arrives third and feeds
    # the AV matmuls (the 6-deep es pool bridges the exp->AV lag).
    # PSUM: projections 4 banks (pool closed before attention), then
    # scores 2x[128,1024] (4 banks) + 4 AV-accumulator banks (the output
    # projection borrows freed accumulator slots).
    with tc.tile_pool(name="xin", bufs=1) as xinp, \
         tc.tile_pool(name="expsb", bufs=8) as expsb, \
         tc.tile_pool(name="otsb", bufs=4) as otsbp, \
         tc.tile_pool(name="recsb", bufs=4) as recp, \
         tc.tile_pool(name="outsb", bufs=2) as outsbp:

        # per-c-block DMAs so projections can stream behind the loads
        xt = {}
        for name, eng in (("xkt", nc.sync), ("xqt", nc.scalar),
                          ("xvt", nc.sync)):
            t_ = xinp.tile([128, NCC * T], bf16, tag=name, name=name)
            for c in range(NCC):
                eng.dma_start(
                    t_[:, c * T:(c + 1) * T],
                    io[name][c * 128:(c + 1) * 128, :])
            xt[name] = t_

        def qk_proj(pool, tag, xname, wname, bname, XT, tci):
            xsb = xt[xname]
            for co in range(2):
                pj = pool.tile([128, TCH], f32, tag=tag, name="proj")
                for c in range(NCC):
                    nc.tensor.matmul(
                        pj[:],
                        lhsT=wsb[wname][c][:, co * 128:(co + 1) * 128],
                        rhs=xsb[:, c * T + tci * TCH:
                                c * T + (tci + 1) * TCH],
                        start=(c == 0), stop=(c == NCC - 1))
                nc.vector.tensor_scalar_add(
                    XT[co][:, tci * TCH:(tci + 1) * TCH],
                    pj[:], bias[bname][:, co:co + 1])

        with tc.tile_pool(name="pr", bufs=4, space="PSUM") as projps:
            for tci in range(NTCH):
                qk_proj(projps, "pr", "xkt", "wk", "bks", KT, tci)
            qk_proj(projps, "pr", "xqt", "wq", "bqs", QT, 0)

        sps = ctx.enter_context(tc.tile_pool(name="sps", bufs=2,
                                             space="PSUM"))
        otps = ctx.enter_context(tc.tile_pool(name="ot", bufs=4,
                                              space="PSUM"))

        def v_nat():
            # Vnat[t, ds] = sum_c xvT[c, t].T @ wv[c, ds]
            xv = xt["xvt"]
            bvb3 = bvb[:].rearrange("p (h d) -> p h d", h=HG)
            for tt in range(NKT):
                pv = otps.tile([128, DS], f32, tag="ot", name="vnat")
                for c in range(NCC):
                    nc.tensor.matmul(
                        pv[:],
                        lhsT=xv[:, c * T + tt * 128:c * T + (tt + 1) * 128],
                        rhs=wsb["wv"][c][:],
                        start=(c == 0), stop=(c == NCC - 1))
                dst3 = VNB[:, tt * 512:(tt + 1) * 512].rearrange(
                    "p (h c) -> p h c", h=HG)[:, :, 0:64]
                src3 = pv[:].rearrange("p (h d) -> p h d", h=HG)
                nc.vector.tensor_add(dst3, src3, bvb3)

        def attention(qc):
            qcols = slice(qc * TCH, (qc + 1) * TCH)
            ot_sb = []
            for pr in range(2):
                otp = [otps.tile([128, TCH], f32, tag="ot", name="ot")
                       for _ in range(2)]
                for kt in range(NKT):
                    first, last = kt == 0, kt == NKT - 1
                    S = sps.tile([128, 2 * TCH], f32, tag="s", name="s")
                    for hh in range(2):
                        rows = slice(hh * 64, (hh + 1) * 64)
                        nc.tensor.matmul(
                            S[:, hh * TCH:(hh + 1) * TCH],
                            lhsT=KT[pr][rows, kt * 128:(kt + 1) * 128],
                            rhs=QT[pr][rows, qcols],
                            start=True, stop=True)
                    es = expsb.tile([128, 2 * TCH], bf16, tag="es",
                                    name="es")
                    nc.scalar.activation(es[:], S[:], EXP, scale=SCALE)
                    for hh in range(2):
                        h = pr * 2 + hh
                        nc.tensor.matmul(
                            otp[hh][:, :],
                            lhsT=VNB[:, kt * 512 + h * 128:
                                     kt * 512 + (h + 1) * 128],
                            rhs=es[:, hh * TCH:(hh + 1) * TCH],
                            start=first, stop=last)
                # normalize: psum rows 64-127 hold the denominator
                osb = otsbp.tile([128, TCH], bf16, tag="otsb",
                                 name="otsb")
                for hh in range(2):
                    rec = recp.tile([64, TCH], f32, tag="rec", name="rec")
                    nc.vector.reciprocal(rec[:], otp[hh][64:128, :])
                    nc.vector.tensor_mul(
                        osb[hh * 64:(hh + 1) * 64, :],
                        otp[hh][0:64, :], rec[:])
                ot_sb.append(osb)
            ob = outsbp.tile([128, NCC * TCH], bf16, tag="ob", name="ob")
            for ct in range(NCC):
                pp = otps.tile([128, TCH], f32, tag="ot", name="prj")
                for dc in range(2):
                    nc.tensor.matmul(
                        pp[:],
                        lhsT=wot[dc][:, ct * 128:(ct + 1) * 128],
                        rhs=ot_sb[dc][:],
                        start=(dc == 0), stop=(dc == 1))
                nc.vector.tensor_copy(
                    ob[:, ct * TCH:(ct + 1) * TCH], pp[:])
                if ct == NCC // 2 - 1 or ct == NCC - 1:
                    lo = 0 if ct < NCC // 2 else NCC // 2
                    nc.sync.dma_start(
                        io["out_t"][lo * 128:(ct + 1) * 128, qcols]
                        .rearrange("(a p) t -> p a t", p=128),
                        ob[:, lo * TCH:(ct + 1) * TCH]
                        .rearrange("p (a t) -> p a t", a=NCC // 2))

        v_nat()
        attention(0)
        for qc in range(1, NTCH):
            qk_proj(otps, "ot", "xqt", "wq", "bqs", QT, qc)
            attention(qc)


def build_nc(reps=1):
    from contextlib import ExitStack

    import concourse.tile as tile
    from concourse import bacc, mybir

    f32 = mybir.dt.float32
    bf16 = mybir.dt.bfloat16
    nc = bacc.Bacc("TRN2", target_bir_lowering=False, debug=False,
                   num_devices=NCORES)
    io = {}
    for name in ("xqt", "xkt", "xvt"):
        io[name] = nc.dram_tensor(name, [C, T], bf16,
                                  kind="ExternalInput").ap()
    for name in ("wq", "wk", "wv"):
        io[name] = nc.dram_tensor(name, [C, DS], bf16,
                                  kind="ExternalInput").ap()
    io["wot"] = nc.dram_tensor("wot", [DS, C], bf16, kind="ExternalInput").ap()
    for name in ("bqs", "bks"):
        io[name] = nc.dram_tensor(name, [DS, 1], f32, kind="ExternalInput").ap()
    io["bvb"] = nc.dram_tensor("bvb", [128, DS], f32, kind="ExternalInput").ap()
    io["out_t"] = nc.dram_tensor("out_t", [C, T], bf16,
                                 kind="ExternalOutput").ap()

    with tile.TileContext(nc) as tc:
        if reps == 1:
            with ExitStack() as ctx:
                _emit(ctx, tc, io)
        else:
            with tc.For_i(0, reps, 1):
                with ExitStack() as ctx:
                    _emit(ctx, tc, io)
    nc.compile()
    return nc


def get_nc():
    global _NC_CACHE
    if _NC_CACHE is None:
        _NC_CACHE = build_nc()
    return _NC_CACHE


def make_in_maps(q, k, v, Wq, bq, Wk, bk, Wv, bv, Wo, bo):
    import ml_dtypes

    bfdt = ml_dtypes.bfloat16
    q, k, v = (np.asarray(x, np.float32) for x in (q, k, v))
    Wq, Wk, Wv, Wo = (np.asarray(x, np.float32) for x in (Wq, Wk, Wv, Wo))
    bq, bk, bv, bo = (np.asarray(x, np.float32) for x in (bq, bk, bv, bo))
    # shared per-batch transposed activations (shared across 4 cores)
    xqt = [np.ascontiguousarray(q[b].T).astype(bfdt) for b in range(B)]
    xkt = [np.ascontiguousarray(k[b].T).astype(bfdt) for b in range(B)]
    xvt = [np.ascontiguousarray(v[b].T).astype(bfdt) for b in range(B)]
    in_maps = []
    for core in range(NCORES):
        b, g = divmod(core, GROUPS)
        sl = slice(g * DS, (g + 1) * DS)
        in_maps.append({
            "xqt": xqt[b],
            "xkt": xkt[b],
            "xvt": xvt[b],
            "wq": np.ascontiguousarray(Wq[sl, :].T).astype(bfdt),
            "wk": np.ascontiguousarray(Wk[sl, :].T).astype(bfdt),
            "wv": np.ascontiguousarray(Wv[sl, :].T).astype(bfdt),
            "wot": np.ascontiguousarray(Wo[:, sl].T).astype(bfdt),
            "bqs": np.ascontiguousarray(bq[sl].reshape(DS, 1)),
            "bks": np.ascontiguousarray(bk[sl].reshape(DS, 1)),
            "bvb": np.ascontiguousarray(
                np.broadcast_to(bv[sl], (128, DS))).astype(np.float32),
        })
    return in_maps


def combine(results, bo):
    out = np.zeros((B, T, C), np.float32)
    for core in range(NCORES):
        b, _ = divmod(core, GROUPS)
        out[b] += results[core]["out_t"].T.astype(np.float32)
    out += np.asarray(bo, np.float32)
    return out


def kernel(q, k, v, Wq, bq, Wk, bk, Wv, bv, Wo, bo):
    from concourse.bass_utils import run_bass_kernel_spmd

    nc = get_nc()
    in_maps = make_in_maps(q, k, v, Wq, bq, Wk, bk, Wv, bv, Wo, bo)
    res = run_bass_kernel_spmd(nc, in_maps, core_ids=list(range(NCORES)))
    return combine(res.results, bo)


# revision 18
# speedup vs baseline: 22.7233x; 14.7543x over previous
"""MultiHeadAttention Trainium2 kernel (8-core SPMD).

Problem: B=2, T=2048, C=1024, H=16 heads, D=64.
  out = softmax((q Wq^T + bq)(k Wk^T + bk)^T / sqrt(D)) (v Wv^T + bv) Wo^T + bo

Sharding: core c -> (batch b = c // 4, head-group g = c % 4).  Each core
computes 4 heads (a 256-wide slice of the projection space) of one batch
element, including its partial contribution to the row-sharded output
projection.  The host sums the 4 partial outputs per batch and adds bo.

v2 design vs v1:
  - Activations are pre-transposed AND pre-cast to bf16 on the HOST:
    xqt/xkt/xvt = x[b].T as [C, T] bf16.  No PE transposes, no PSUM->SBUF
    staging copies, and half the input DMA bytes.
  - Q/K projections: QT/KT[co, t] = W^T.T @ xT directly (bf16 matmul,
    fp32 psum), stored f32r with bias folded via DVE tensor_scalar_add.
  - V projected in NATURAL orientation (tokens on partitions):
    Vnat[t, ds] = xvT_tile.T @ Wv^T — feeds AV without any transpose.
    VN layout per key-tile: [4 heads x (64 V cols + 64 ones cols)]; the
    ones columns make each AV matmul also emit the softmax denominator.
  - S^T[k, q] per head pair: two row-packed matmuls (partitions 0:64 /
    64:128 -> concurrent PE row-groups) into one [128, 1024] psum pair;
    ONE exp activation covers both heads (bf16 out, scale=1/8 folded).
  - Output projection accumulates W_o slices over the 256 head dims,
    written bf16; bo is added on the host during the combine.
"""

import numpy as np

B, T, C, H, D = 2, 2048, 1024, 16, 64
NCORES = 8
GROUPS = 4              # head-groups == cores per batch element
HG = H // GROUPS        # heads per core
DS = HG * D             # per-core projection slice width (256)
TCH = 512               # token chunk (psum bank = 512 fp32)
NTCH = T // TCH         # 4
NCC = C // 128          # 8 contraction chunks
NKT = T // 128          # 16 key tiles
SCALE = float(D) ** -0.5

_NC_CACHE = None


def _emit(ctx, tc, io):
    from concourse import mybir

    nc = tc.nc
    f32 = mybir.dt.float32
    f32r = mybir.dt.float32r
    bf16 = mybir.dt.bfloat16
    EXP = mybir.ActivationFunctionType.Exp

    persist = ctx.enter_context(tc.tile_pool(name="persist", bufs=1))

    def ptile(tag, shape, dt=f32):
        return persist.tile(shape, dt, tag=tag, name=tag)

    # --- persistent SBUF tensors ---------------------------------------
    wsb = {}
    for name in ("wq", "wk", "wv"):
        tiles = []
        for c in range(NCC):
            t_ = ptile(f"{name}{c}", [128, DS], bf16)
            nc.scalar.dma_start(t_[:], io[name][c * 128:(c + 1) * 128, :])
            tiles.append(t_)
        wsb[name] = tiles
    wot = []
    for dc in range(2):
        t_ = ptile(f"wot{dc}", [128, C], bf16)
        nc.scalar.dma_start(t_[:], io["wot"][dc * 128:(dc + 1) * 128, :])
        wot.append(t_)

    bias = {}
    for name in ("bqs", "bks"):
        t_ = ptile(name, [128, 2])
        nc.scalar.dma_start(
            t_[:], io[name].rearrange("(a p) o -> p (a o)", p=128))
        bias[name] = t_
    bvb = ptile("bvb", [128, DS])
    nc.scalar.dma_start(bvb[:], io["bvb"][:, :])

    QT = [ptile(f"qt{i}", [128, T], f32r) for i in range(2)]
    KT = [ptile(f"kt{i}", [128, T], f32r) for i in range(2)]
    # V natural, all 16 key tiles in one buffer; per key tile the layout is
    # [4 heads x (64 V cols + 64 ones cols)].  Ones prefilled via memset;
    # V columns overwritten when the V projection lands.
    VNB = ptile("vnb", [128, NKT * 512], bf16)
    nc.gpsimd.memset(VNB[:], 1.0)

    # --- stage A + B, software-pipelined -------------------------------
    # DMA order: weights (scalar), xkt (sync), xqt (scalar), xvt (sync):
    # scores for q-chunk 0 only need K fully projected + Q chunk 0, so the
    # exp stream starts as soon as K + Q0 land; V arrives third and feeds
    # the AV matmuls (the 6-deep es pool bridges the exp->AV lag).
    # PSUM: projections 4 banks (pool closed before attention), then
    # scores 2x[128,1024] (4 banks) + 4 AV-accumulator banks (the output
    # projection borrows freed accumulator slots).
    with tc.tile_pool(name="xin", bufs=1) as xinp, \
         tc.tile_pool(name="expsb", bufs=8) as expsb, \
         tc.tile_pool(name="otsb", bufs=4) as otsbp, \
         tc.tile_pool(name="recsb", bufs=4) as recp, \
         tc.tile_pool(name="outsb", bufs=2) as outsbp:

        # per-c-block DMAs so projections can stream behind the loads;
        # xvt split across both queues so it lands soon after xqt
        xt = {}
        for name, engs in (("xkt", (nc.sync,)), ("xqt", (nc.scalar,)),
                           ("xvt", (nc.sync, nc.scalar))):
            t_ = xinp.tile([128, NCC * T], bf16, tag=name, name=name)
            for c in range(NCC):
                engs[c % len(engs)].dma_start(
                    t_[:, c * T:(c + 1) * T],
                    io[name][c * 128:(c + 1) * 128, :])
            xt[name] = t_

        def qk_proj(pool, tag, xname, wname, bname, XT, tci):
            xsb = xt[xname]
            for co in range(2):
                pj = pool.tile([128, TCH], f32, tag=tag, name="proj")
                for c in range(NCC):
                    nc.tensor.matmul(
                        pj[:],
                        lhsT=wsb[wname][c][:, co * 128:(co + 1) * 128],
                        rhs=xsb[:, c * T + tci * TCH:
                                c * T + (tci + 1) * TCH],
                        start=(c == 0), stop=(c == NCC - 1))
                nc.vector.tensor_scalar_add(
                    XT[co][:, tci * TCH:(tci + 1) * TCH],
                    pj[:], bias[bname][:, co:co + 1])

        with tc.tile_pool(name="pr", bufs=4, space="PSUM") as projps:
            for tci in range(NTCH):
                qk_proj(projps, "pr", "xkt", "wk", "bks", KT, tci)
            qk_proj(projps, "pr", "xqt", "wq", "bqs", QT, 0)

        sps = ctx.enter_context(tc.tile_pool(name="sps", bufs=2,
                                             space="PSUM"))
        otps = ctx.enter_context(tc.tile_pool(name="ot", bufs=4,
                                              space="PSUM"))

        def v_nat_tile(tt):
            # Vnat[t, ds] = sum_c xvT[c, t].T @ wv[c, ds]
            xv = xt["xvt"]
            bvb3 = bvb[:].rearrange("p (h d) -> p h d", h=HG)
            pv = otps.tile([128, DS], f32, tag="ot", name="vnat")
            for c in range(NCC):
                nc.tensor.matmul(
                    pv[:],
                    lhsT=xv[:, c * T + tt * 128:c * T + (tt + 1) * 128],
                    rhs=wsb["wv"][c][:],
                    start=(c == 0), stop=(c == NCC - 1))
            dst3 = VNB[:, tt * 512:(tt + 1) * 512].rearrange(
                "p (h c) -> p h c", h=HG)[:, :, 0:64]
            src3 = pv[:].rearrange("p (h d) -> p h d", h=HG)
            nc.vector.tensor_add(dst3, src3, bvb3)

        def attention(qc, interleave=None):
            # interleave: kt -> callable, emitted inside the pr0 loop so the
            # static schedule folds extra PE work under the exp stream
            qcols = slice(qc * TCH, (qc + 1) * TCH)
            ot_sb = []
            for pr in range(2):
                otp = [otps.tile([128, TCH], f32, tag="ot", name="ot")
                       for _ in range(2)]
                for kt in range(NKT):
                    first, last = kt == 0, kt == NKT - 1
                    S = sps.tile([128, 2 * TCH], f32, tag="s", name="s")
                    for hh in range(2):
                        rows = slice(hh * 64, (hh + 1) * 64)
                        nc.tensor.matmul(
                            S[:, hh * TCH:(hh + 1) * TCH],
                            lhsT=KT[pr][rows, kt * 128:(kt + 1) * 128],
                            rhs=QT[pr][rows, qcols],
                            start=True, stop=True)
                    es = expsb.tile([128, 2 * TCH], bf16, tag="es",
                                    name="es")
                    nc.scalar.activation(es[:], S[:], EXP, scale=SCALE)
                    if interleave is not None:
                        interleave(pr, kt)
                    for hh in range(2):
                        h = pr * 2 + hh
                        nc.tensor.matmul(
                            otp[hh][:, :],
                            lhsT=VNB[:, kt * 512 + h * 128:
                                     kt * 512 + (h + 1) * 128],
                            rhs=es[:, hh * TCH:(hh + 1) * TCH],
                            start=first, stop=last)
                # normalize: psum rows 64-127 hold the denominator
                osb = otsbp.tile([128, TCH], bf16, tag="otsb",
                                 name="otsb")
                for hh in range(2):
                    rec = recp.tile([64, TCH], f32, tag="rec", name="rec")
                    nc.vector.reciprocal(rec[:], otp[hh][64:128, :])
                    nc.vector.tensor_mul(
                        osb[hh * 64:(hh + 1) * 64, :],
                        otp[hh][0:64, :], rec[:])
                ot_sb.append(osb)
            ob = outsbp.tile([128, NCC * TCH], bf16, tag="ob", name="ob")
            for ct in range(NCC):
                pp = otps.tile([128, TCH], f32, tag="ot", name="prj")
                for dc in range(2):
                    nc.tensor.matmul(
                        pp[:],
                        lhsT=wot[dc][:, ct * 128:(ct + 1) * 128],
                        rhs=ot_sb[dc][:],
                        start=(dc == 0), stop=(dc == 1))
                nc.vector.tensor_copy(
                    ob[:, ct * TCH:(ct + 1) * TCH], pp[:])
                if ct == NCC // 2 - 1 or ct == NCC - 1:
                    lo = 0 if ct < NCC // 2 else NCC // 2
                    nc.sync.dma_start(
                        io["out_t"][lo * 128:(ct + 1) * 128, qcols]
                        .rearrange("(a p) t -> p a t", p=128),
                        ob[:, lo * TCH:(ct + 1) * TCH]
                        .rearrange("p (a t) -> p a t", a=NCC // 2))

        def make_interleave(qc):
            # fold V-nat (qc0/pr0) and the NEXT q-chunk's projection
            # (pr1, two co halves at kt 4/10) under this chunk's exp stream
            def il(pr, kt):
                if qc == 0 and pr == 0:
                    v_nat_tile(kt)
                elif pr == 1 and qc < NTCH - 1 and kt in (4, 10):
                    co = 0 if kt == 4 else 1
                    xsb = xt["xqt"]
                    tci = qc + 1
                    pj = otps.tile([128, TCH], f32, tag="ot", name="proj")
                    for c in range(NCC):
                        nc.tensor.matmul(
                            pj[:],
                            lhsT=wsb["wq"][c][:, co * 128:(co + 1) * 128],
                            rhs=xsb[:, c * T + tci * TCH:
                                    c * T + (tci + 1) * TCH],
                            start=(c == 0), stop=(c == NCC - 1))
                    nc.vector.tensor_scalar_add(
                        QT[co][:, tci * TCH:(tci + 1) * TCH],
                        pj[:], bias["bqs"][:, co:co + 1])
            return il

        for qc in range(NTCH):
            attention(qc, interleave=make_interleave(qc))


def build_nc(reps=1):
    from contextlib import ExitStack

    import concourse.tile as tile
    from concourse import bacc, mybir

    f32 = mybir.dt.float32
    bf16 = mybir.dt.bfloat16
    nc = bacc.Bacc("TRN2", target_bir_lowering=False, debug=False,
                   num_devices=NCORES)
    io = {}
    for name in ("xqt", "xkt", "xvt"):
        io[name] = nc.dram_tensor(name, [C, T], bf16,
                                  kind="ExternalInput").ap()
    for name in ("wq", "wk", "wv"):
        io[name] = nc.dram_tensor(name, [C, DS], bf16,
                                  kind="ExternalInput").ap()
    io["wot"] = nc.dram_tensor("wot", [DS, C], bf16, kind="ExternalInput").ap()
    for name in ("bqs", "bks"):
        io[name] = nc.dram_tensor(name, [DS, 1], f32, kind="ExternalInput").ap()
    io["bvb"] = nc.dram_tensor("bvb", [128, DS], f32, kind="ExternalInput").ap()
    io["out_t"] = nc.dram_tensor("out_t", [C, T], bf16,
                                 kind="ExternalOutput").ap()

    with tile.TileContext(nc) as tc:
        if reps == 1:
            with ExitStack() as ctx:
                _emit(ctx, tc, io)
        else:
            with tc.For_i(0, reps, 1):
                with ExitStack() as ctx:
                    _emit(ctx, tc, io)
    nc.compile()
    return nc


def get_nc():
    global _NC_CACHE
    if _NC_CACHE is None:
        _NC_CACHE = build_nc()
    return _NC_CACHE


def make_in_maps(q, k, v, Wq, bq, Wk, bk, Wv, bv, Wo, bo):
    import ml_dtypes

    bfdt = ml_dtypes.bfloat16
    q, k, v = (np.asarray(x, np.float32) for x in (q, k, v))
    Wq, Wk, Wv, Wo = (np.asarray(x, np.float32) for x in (Wq, Wk, Wv, Wo))
    bq, bk, bv, bo = (np.asarray(x, np.float32) for x in (bq, bk, bv, bo))
    # shared per-batch transposed activations (shared across 4 cores)
    xqt = [np.ascontiguousarray(q[b].T).astype(bfdt) for b in range(B)]
    xkt = [np.ascontiguousarray(k[b].T).astype(bfdt) for b in range(B)]
    xvt = [np.ascontiguousarray(v[b].T).astype(bfdt) for b in range(B)]
    in_maps = []
    for core in range(NCORES):
        b, g = divmod(core, GROUPS)
        sl = slice(g * DS, (g + 1) * DS)
        in_maps.append({
            "xqt": xqt[b],
            "xkt": xkt[b],
            "xvt": xvt[b],
            "wq": np.ascontiguousarray(Wq[sl, :].T).astype(bfdt),
            "wk": np.ascontiguousarray(Wk[sl, :].T).astype(bfdt),
            "wv": np.ascontiguousarray(Wv[sl, :].T).astype(bfdt),
            "wot": np.ascontiguousarray(Wo[:, sl].T).astype(bfdt),
            "bqs": np.ascontiguousarray(bq[sl].reshape(DS, 1)),
            "bks": np.ascontiguousarray(bk[sl].reshape(DS, 1)),
            "bvb": np.ascontiguousarray(
                np.broadcast_to(bv[sl], (128, DS))).astype(np.float32),
        })
    return in_maps


def combine(results, bo):
    out = np.zeros((B, T, C), np.float32)
    for core in range(NCORES):
        b, _ = divmod(core, GROUPS)
        out[b] += results[core]["out_t"].T.astype(np.float32)
    out += np.asarray(bo, np.float32)
    return out


def kernel(q, k, v, Wq, bq, Wk, bk, Wv, bv, Wo, bo):
    from concourse.bass_utils import run_bass_kernel_spmd

    nc = get_nc()
    in_maps = make_in_maps(q, k, v, Wq, bq, Wk, bk, Wv, bv, Wo, bo)
    res = run_bass_kernel_spmd(nc, in_maps, core_ids=list(range(NCORES)))
    return combine(res.results, bo)
